# revision 49
# baseline (speedup 1.0000x reference)
"""Sparse-attention (2D RoPE + softmax attention) Trainium2 Bass kernel.

Problem: B=8, H=8, N=1024 (32x32 grid), D=256 per head, fp32 I/O.
Sharding: B*H = 64 heads split across 8 NeuronCores (8 heads/core),
no cross-core communication.

Per-head pipeline on each core (1-deep software pipeline across heads):
  1. gpsimd cast-DMA: Q/K/V fp32 DRAM -> bf16 SBUF (natural [tok, d] layout,
     Q/K as two half-tensors for finer availability)
  2. PE transpose (bf16): stride-2 weight reads split even/odd d (RoPE pair
     components) while transposing [tok, i] blocks -> [pair i, tok] D-major
     tiles in PSUM
  3. DVE RoPE: QR0 = A*cos - B*sin, QR1 = A*sin + B*cos  (bf16, 2x mode)
  4. PE scores: ST[m,n] = sum_d KRT[d,m] QRT[d,n]  (S-transposed layout)
  5. ACT exp(scale=1/16): PSUM fp32 -> bf16 P~ tiles (no max subtraction:
     scores ~ N(0,1), max < ~6, exp is safe)
  6. PE PV: out[n,d] = sum_m P~[m,n] V_aug[m,d]; V augmented with a ones
     column so column 256 accumulates the softmax denominator
  7. DVE reciprocal + tensor_scalar normalize -> fp32 out, DMA store

The d-axis of QRT/KRT is in deinterleaved (even dims | odd dims) order for
both Q and K; scores are invariant to any shared permutation of d.

Fill-phase scheduling (head 0): the PE HAM clock gate re-throttles to
1.2 GHz after ~3.4us of low activity, and PE transposes don't count as
activity.  Head 0's DVE rope latency (~4-8us) between the transposes and
the first score matmuls left exactly such a window in the v1 kernel -- the
whole first ~10us of real score work ran at half clock.  v2 bridges the
gap with dense 128-free filler matmuls (F0 before the transposes, F1/F2
pinned after the lo/hi chunk transposes via SBUF data deps), orders head-0
rope as q-lo, k-lo, k-hi, q-hi, and emits head-0 scores in four mb/nch
quarters so they start as soon as each rope slice lands.
"""

import sys

for _p in ("/opt/trn_rl_repo", "/opt/pypackages"):
    if _p not in sys.path:
        sys.path.insert(0, _p)

import numpy as np
import ml_dtypes

GRID = 32
DIM = 256
PAIRS = DIM // 2  # 128
N = GRID * GRID  # 1024
NB = N // 128  # 8 token blocks
B, H = 8, 8
NCORES = 8
HPC = (B * H) // NCORES  # heads per core

# Filler matmul counts bridging the head-0 fill phase (tuned on traces).
NF0 = 60  # pure warmup until head-0 q-lo is cast to bf16 (~12.7us)
NF1 = 22  # between the Q-lo and K-lo transposes
NF2 = 38  # after the Q-hi transposes, while DVE ropes k-lo/q-hi


def rope_tables():
    """cosT/sinT in transposed layout [pair i, token t], bf16."""
    dim_half = DIM // 2
    inv = 1.0 / (10000.0 ** (np.arange(0, dim_half, 2).astype(np.float32) / dim_half))
    fx = np.outer(np.arange(GRID, dtype=np.float32), inv)  # (32, 64) by x
    fy = np.outer(np.arange(GRID, dtype=np.float32), inv)  # (32, 64) by y
    # token t = y*32 + x ; ang[t, i<64] = fx[x, i]; ang[t, i>=64] = fy[y, i-64]
    fx_grid = np.broadcast_to(fx[None, :, :], (GRID, GRID, fx.shape[1]))
    fy_grid = np.broadcast_to(fy[:, None, :], (GRID, GRID, fy.shape[1]))
    ang = np.concatenate([fx_grid, fy_grid], axis=-1).reshape(N, dim_half)
    cosT = np.ascontiguousarray(np.cos(ang).T).astype(ml_dtypes.bfloat16)
    sinT = np.ascontiguousarray(np.sin(ang).T).astype(ml_dtypes.bfloat16)
    return cosT, sinT


def build(n_heads=HPC):
    """Build the Bass program for one core processing n_heads heads."""
    import concourse.mybir as mybir
    import concourse.tile as tile
    from concourse import bacc
    from concourse.masks import make_identity

    bf16 = mybir.dt.bfloat16
    f32 = mybir.dt.float32
    Exp = mybir.ActivationFunctionType.Exp

    nc = bacc.Bacc(None, target_bir_lowering=False)
    names = {}

    with tile.TileContext(nc) as tc:
        with tc.tile_pool(name="dram", bufs=1, space="DRAM") as dram:
            Qd = dram.tile([n_heads, N, DIM], f32, kind="ExternalInput", name="Q")
            Kd = dram.tile([n_heads, N, DIM], f32, kind="ExternalInput", name="K")
            Vd = dram.tile([n_heads, N, DIM], f32, kind="ExternalInput", name="V")
            Cd = dram.tile([PAIRS, N], bf16, kind="ExternalInput", name="COS")
            Sd = dram.tile([PAIRS, N], bf16, kind="ExternalInput", name="SIN")
            Od = dram.tile([n_heads, N, DIM], f32, kind="ExternalOutput", name="OUT")
        names = {k: v.name for k, v in
                 dict(Q=Qd, K=Kd, V=Vd, COS=Cd, SIN=Sd, OUT=Od).items()}

        with (
            tc.tile_pool(name="const", bufs=1) as constp,
            tc.tile_pool(name="f32s", bufs=2) as f32p,
            tc.tile_pool(name="nat", bufs=12) as natp,
            tc.tile_pool(name="rt", bufs=6) as rtp,
            tc.tile_pool(name="tmp", bufs=4) as tmpp,
            tc.tile_pool(name="pt", bufs=3) as ptp,
            tc.tile_pool(name="va", bufs=3) as vap,
            tc.tile_pool(name="osb", bufs=3) as osbp,
            tc.tile_pool(name="rcp", bufs=8) as rcpp,
            tc.tile_pool(name="ptr", bufs=1, space="PSUM") as trp,
            tc.tile_pool(name="pst", bufs=3, space="PSUM") as stp,
            tc.tile_pool(name="pov", bufs=3, space="PSUM") as povp,
        ):
            # identity for PE transposes: first thing on gpsimd so the
            # head-0 cast-DMA issues right behind it
            ident = constp.tile([128, 128], bf16, name="ident")
            make_identity(nc, ident)

            NBH = NB // 2  # token blocks per load chunk

            def load_chunk(src, h, c):
                """Issue one half-tensor cast-DMA; returns pair-component
                view [p, nb, i, two]."""
                view = src[h].rearrange("(c nb p) d -> p c nb d", p=128, c=2)
                t = natp.tile([128, NBH, DIM], bf16, name="nh", tag="nat")
                nc.gpsimd.dma_start(t, view[:, c])
                return t

            def load_v(h):
                va = vap.tile([128, NB, DIM + 1], bf16, name="va", tag="va")
                nc.gpsimd.dma_start(
                    va[:, :, 0:DIM], Vd[h].rearrange("(mb p) d -> p mb d", p=128)
                )
                nc.vector.memset(va[:, :, DIM : DIM + 1], 1.0)
                return va

            # head-0 Q/K chunk loads go out first in consumption order
            # (q-lo, k-lo, q-hi, k-hi); every gpsimd cast-DMA completes
            # ~6us after issue (fixed SWDGE latency), contention or not.
            # cos/sin ride the otherwise-idle scalar HWDGE queue -- on the
            # sync queue behind other traffic they arrive ~16us and gate
            # the entire rope chain.
            # Head-0's lo chunks -- which head the serial DVE rope chain --
            # bypass the 6-8us SWDGE cast latency: fp32 over the HWDGE
            # queues (q-lo on sync, k-lo on scalar AFTER the rope tables so
            # the tables are never starved), cast bf16 on the idle ACT
            # engine in consumption order.  Hi chunks and V keep gpsimd.
            q_chunks = [None, None]
            k_chunks = [None, None]
            q32 = f32p.tile([128, NBH, DIM], f32, name="q32", tag="f32s")
            nc.sync.dma_start(
                q32, Qd[0].rearrange("(c nb p) d -> p c nb d", p=128, c=2)[:, 0]
            )
            q_chunks[1] = load_chunk(Qd, 0, 1)
            k_chunks[1] = load_chunk(Kd, 0, 1)
            va0 = load_v(0)

            cosT = constp.tile([128, N], bf16, name="cosT")
            sinT = constp.tile([128, N], bf16, name="sinT")
            nc.scalar.dma_start(cosT, Cd[:])
            nc.scalar.dma_start(sinT, Sd[:])
            k32 = f32p.tile([128, NBH, DIM], f32, name="k32", tag="f32s")
            nc.scalar.dma_start(
                k32, Kd[0].rearrange("(c nb p) d -> p c nb d", p=128, c=2)[:, 0]
            )
            # ACT casts, q-lo then k-lo (exp activations come much later)
            q_chunks[0] = natp.tile([128, NBH, DIM], bf16, name="nh", tag="nat")
            nc.scalar.copy(q_chunks[0], q32)
            k_chunks[0] = natp.tile([128, NBH, DIM], bf16, name="nh", tag="nat")
            nc.scalar.copy(k_chunks[0], k32)

            # filler data for HAM-warming matmuls (DVE memset, t~0).
            # wupsum shares the st rotation: all fillers finish before the
            # first score matmul needs that bank (WAR dep, by construction).
            wudata = constp.tile([128, 128], bf16, name="wudata")
            nc.vector.memset(wudata, 0.5)
            wupsum = stp.tile([128, 512], f32, name="wupsum", tag="st")

            def filler(n, lhs=None, tag=""):
                """n dense 128-free matmuls on the PE. lhs (SBUF bf16
                [128, >=128]) pins placement: the block can't start before
                lhs's producer (DMA) lands, so emission order holds."""
                src = lhs if lhs is not None else wudata
                for i in range(n):
                    nc.tensor.matmul(
                        wupsum[:, 0:128],
                        lhsT=src[:, 0:128] if lhs is not None else wudata,
                        rhs=wudata,
                        start=(i == 0),
                        stop=(i == n - 1),
                    )

            def pair_view(t):
                return t.rearrange("p nb (i two) -> p two nb i", two=2)

            def transpose_chunk(dei, tr_pair, col_base):
                """Transpose one chunk's 4 token blocks, both halves, into
                trA/trB columns [col_base : col_base+512]."""
                trA, trB = tr_pair
                for half in (0, 1):
                    tr = trA if half == 0 else trB
                    for nb in range(NBH):
                        cb = col_base + nb * 128
                        nc.tensor.transpose(
                            tr[:, cb : cb + 128],
                            dei[:, half, nb],
                            ident,
                        )

            def rope_slice(rt, tr_pair, src_sl, tok_sl):
                """Apply rotary: rt[:, {0,1}, tok_sl] from trA/trB[:, src_sl]
                (tr columns may hold a different token window than tok_sl)."""
                trA, trB = tr_pair
                W = tok_sl.stop - tok_sl.start
                t1 = tmpp.tile([128, W], bf16, name="t1", tag="tmp")
                t2 = tmpp.tile([128, W], bf16, name="t2", tag="tmp")
                t3 = tmpp.tile([128, W], bf16, name="t3", tag="tmp")
                t4 = tmpp.tile([128, W], bf16, name="t4", tag="tmp")
                nc.vector.tensor_mul(t1, trA[:, src_sl], cosT[:, tok_sl])
                nc.vector.tensor_mul(t2, trB[:, src_sl], sinT[:, tok_sl])
                nc.vector.tensor_sub(rt[:, 0, tok_sl], t1, t2)
                nc.vector.tensor_mul(t3, trA[:, src_sl], sinT[:, tok_sl])
                nc.vector.tensor_mul(t4, trB[:, src_sl], cosT[:, tok_sl])
                nc.vector.tensor_add(rt[:, 1, tok_sl], t3, t4)

            def rope_tensor(dei_chunks, out_name):
                """Steady-state: transpose both chunks then rope full width."""
                rt = rtp.tile([128, 2, N], bf16, name=out_name, tag="rt")
                trA = trp.tile([128, N], bf16, name="trA", tag="trA")
                trB = trp.tile([128, N], bf16, name="trB", tag="trB")
                for c in range(2):
                    transpose_chunk(pair_view(dei_chunks[c]), (trA, trB), c * 512)
                rope_slice(rt, (trA, trB), slice(0, N), slice(0, N))
                return rt

            def scores_quarter(st_tiles, ptile, qrt, krt, nch, mbs):
                """Score matmuls + exp for mb in mbs, one n-chunk."""
                for mb in mbs:
                    st = st_tiles[(nch, mb)] = stp.tile(
                        [128, 512], f32, name="st", tag="st"
                    )
                    for dt_ in (0, 1):
                        nc.tensor.matmul(
                            st,
                            lhsT=krt[:, dt_, mb * 128 : (mb + 1) * 128],
                            rhs=qrt[:, dt_, nch * 512 : (nch + 1) * 512],
                            start=(dt_ == 0),
                            stop=(dt_ == 1),
                        )
                    nc.scalar.activation(
                        ptile[nch][:, mb],
                        st,
                        Exp,
                        scale=1.0 / 16.0,
                    )

            def pv_phase(h, ptiles, va, last):
                osb = osbp.tile([128, NB, DIM], f32, name="osb", tag="osb")
                od_view = Od[h].rearrange("(nb p) d -> p nb d", p=128)
                for nch in range(2):
                    ptile = ptiles[nch]
                    for nb4 in range(4):
                        po = povp.tile([128, DIM + 1], f32, name="po", tag="po")
                        for mb in range(NB):
                            nc.tensor.matmul(
                                po,
                                lhsT=ptile[:, mb, nb4 * 128 : (nb4 + 1) * 128],
                                rhs=va[:, mb],
                                start=(mb == 0),
                                stop=(mb == NB - 1),
                            )
                        r = rcpp.tile([128, 1], f32, name="r", tag="r")
                        nc.vector.reciprocal(r, po[:, DIM : DIM + 1])
                        gnb = nch * 4 + nb4
                        nc.vector.tensor_scalar_mul(osb[:, gnb], po[:, 0:DIM], r)
                        # finer stores on the last head shrink the tail drain
                        if last and gnb >= 6:
                            nc.sync.dma_start(
                                od_view[:, gnb : gnb + 1],
                                osb[:, gnb : gnb + 1],
                            )
                    if not last:
                        # store per chunk so the final transfer is half-sized
                        nc.sync.dma_start(
                            od_view[:, nch * 4 : (nch + 1) * 4],
                            osb[:, nch * 4 : (nch + 1) * 4],
                        )
                    elif nch == 0:
                        nc.sync.dma_start(od_view[:, 0:4], osb[:, 0:4])
                    else:
                        nc.sync.dma_start(od_view[:, 4:6], osb[:, 4:6])

            def attention(h, qrt, krt, va, split0=False):
                """Scores+exp then PV.  split0 emits head-0's scores in
                availability order (rope order: q-lo, k-lo, k-hi, q-hi)."""
                ptiles = [
                    ptp.tile([128, NB, 512], bf16, name="ptile", tag="pt")
                    for _ in range(2)
                ]
                st_tiles = {}
                if split0:
                    # availability order under rope q-lo, k-lo, q-hi, k-hi
                    scores_quarter(st_tiles, ptiles, qrt, krt, 0, range(0, 4))
                    scores_quarter(st_tiles, ptiles, qrt, krt, 1, range(0, 4))
                    scores_quarter(st_tiles, ptiles, qrt, krt, 0, range(4, 8))
                    scores_quarter(st_tiles, ptiles, qrt, krt, 1, range(4, 8))
                else:
                    scores_quarter(st_tiles, ptiles, qrt, krt, 0, range(8))
                    scores_quarter(st_tiles, ptiles, qrt, krt, 1, range(8))
                pv_phase(h, ptiles, va, last=(h == n_heads - 1))

            # ---- head 0: hand-scheduled fill ----
            # trA/trB columns are shared: [0:512] holds the current Q token
            # chunk, [512:1024] the current K token chunk.  Chunk 1's
            # transposes WAR-wait on chunk 0's rope reads, which is exactly
            # the availability order anyway.
            qrt0 = rtp.tile([128, 2, N], bf16, name="qrt", tag="rt")
            krt0 = rtp.tile([128, 2, N], bf16, name="krt", tag="rt")
            tr0 = (
                trp.tile([128, N], bf16, name="trA", tag="trA"),
                trp.tile([128, N], bf16, name="trB", tag="trB"),
            )
            # DVE rope order: q-lo, k-lo, q-hi, k-hi; hi-chunk transposes
            # WAR-wait on the lo rope reads of the shared tr columns, so
            # each rope_slice must be emitted before the transpose that
            # recycles its source columns.
            filler(NF0)
            transpose_chunk(pair_view(q_chunks[0]), tr0, 0)
            filler(NF1, lhs=q_chunks[0][:, 0, 0:128])
            transpose_chunk(pair_view(k_chunks[0]), tr0, 512)
            rope_slice(qrt0, tr0, slice(0, 512), slice(0, 512))
            rope_slice(krt0, tr0, slice(512, 1024), slice(0, 512))
            transpose_chunk(pair_view(q_chunks[1]), tr0, 0)
            filler(NF2, lhs=q_chunks[1][:, 0, 0:128])
            rope_slice(qrt0, tr0, slice(0, 512), slice(512, 1024))
            transpose_chunk(pair_view(k_chunks[1]), tr0, 512)
            rope_slice(krt0, tr0, slice(512, 1024), slice(512, 1024))

            pending = (0, qrt0, krt0, va0, True)

            # ---- heads 1..n-1: steady-state pipeline ----
            for h in range(1, n_heads + 1):
                if h < n_heads:
                    qc = [load_chunk(Qd, h, c) for c in range(2)]
                    kc = [load_chunk(Kd, h, c) for c in range(2)]
                    va = load_v(h)
                    qrt = rope_tensor(qc, "qrt")
                    krt = rope_tensor(kc, "krt")
                    cur = (h, qrt, krt, va, False)
                else:
                    cur = None
                if pending is not None:
                    attention(*pending[:4], split0=pending[4])
                pending = cur
            if pending is not None:
                attention(*pending[:4], split0=pending[4])

    nc.compile()
    return nc, names


_CACHE = {}


def _get_nc(n_heads=HPC):
    if n_heads not in _CACHE:
        _CACHE[n_heads] = build(n_heads)
    return _CACHE[n_heads]


def _run(Q, K, V, **spmd_kwargs):
    from concourse.bass_utils import run_bass_kernel_spmd

    nc, names = _get_nc(HPC)
    cosT, sinT = rope_tables()
    Qr = np.ascontiguousarray(Q.reshape(B * H, N, DIM), dtype=np.float32)
    Kr = np.ascontiguousarray(K.reshape(B * H, N, DIM), dtype=np.float32)
    Vr = np.ascontiguousarray(V.reshape(B * H, N, DIM), dtype=np.float32)
    in_maps = []
    for c in range(NCORES):
        sl = slice(c * HPC, (c + 1) * HPC)
        in_maps.append(
            {
                names["Q"]: np.ascontiguousarray(Qr[sl]),
                names["K"]: np.ascontiguousarray(Kr[sl]),
                names["V"]: np.ascontiguousarray(Vr[sl]),
                names["COS"]: cosT,
                names["SIN"]: sinT,
            }
        )
    res = run_bass_kernel_spmd(nc, in_maps, core_ids=list(range(NCORES)), **spmd_kwargs)
    out = np.concatenate([r[names["OUT"]] for r in res.results], axis=0)
    return np.ascontiguousarray(out.reshape(B, H, N, DIM), dtype=np.float32), res


def kernel(Q, K, V):
    return _run(Q, K, V)[0]


if __name__ == "__main__":
    rng = np.random.default_rng(0)
    Q = rng.standard_normal((B, H, N, DIM), dtype=np.float32)
    K = rng.standard_normal((B, H, N, DIM), dtype=np.float32)
    V = rng.standard_normal((B, H, N, DIM), dtype=np.float32)
    out = kernel(Q, K, V)
    print("out", out.shape, out.dtype, float(np.abs(out).mean()))


# revision 50
# speedup vs baseline: 1.0267x; 1.0267x over previous
"""Sparse-attention (2D RoPE + softmax attention) Trainium2 Bass kernel.

Problem: B=8, H=8, N=1024 (32x32 grid), D=256 per head, fp32 I/O.
Sharding: B*H = 64 heads split across 8 NeuronCores (8 heads/core),
no cross-core communication.

Per-head pipeline on each core (1-deep software pipeline across heads):
  1. gpsimd cast-DMA: Q/K/V fp32 DRAM -> bf16 SBUF (natural [tok, d] layout,
     Q/K as two half-tensors for finer availability)
  2. PE transpose (bf16): stride-2 weight reads split even/odd d (RoPE pair
     components) while transposing [tok, i] blocks -> [pair i, tok] D-major
     tiles in PSUM
  3. DVE RoPE: QR0 = A*cos - B*sin, QR1 = A*sin + B*cos  (bf16, 2x mode)
  4. PE scores: ST[m,n] = sum_d KRT[d,m] QRT[d,n]  (S-transposed layout)
  5. ACT exp(scale=1/16): PSUM fp32 -> bf16 P~ tiles (no max subtraction:
     scores ~ N(0,1), max < ~6, exp is safe)
  6. PE PV: out[n,d] = sum_m P~[m,n] V_aug[m,d]; V augmented with a ones
     column so column 256 accumulates the softmax denominator
  7. DVE reciprocal + tensor_scalar normalize -> fp32 out, DMA store

The d-axis of QRT/KRT is in deinterleaved (even dims | odd dims) order for
both Q and K; scores are invariant to any shared permutation of d.

Fill-phase scheduling (head 0): the PE HAM clock gate re-throttles to
1.2 GHz after ~3.4us of low activity, and PE transposes don't count as
activity.  Head 0's DVE rope latency (~4-8us) between the transposes and
the first score matmuls left exactly such a window in the v1 kernel -- the
whole first ~10us of real score work ran at half clock.  v2 bridges the
gap with dense 128-free filler matmuls (F0 before the transposes, F1/F2
pinned after the lo/hi chunk transposes via SBUF data deps), orders head-0
rope as q-lo, k-lo, k-hi, q-hi, and emits head-0 scores in four mb/nch
quarters so they start as soon as each rope slice lands.
"""

import sys

for _p in ("/opt/trn_rl_repo", "/opt/pypackages"):
    if _p not in sys.path:
        sys.path.insert(0, _p)

import numpy as np
import ml_dtypes

GRID = 32
DIM = 256
PAIRS = DIM // 2  # 128
N = GRID * GRID  # 1024
NB = N // 128  # 8 token blocks
B, H = 8, 8
NCORES = 8
HPC = (B * H) // NCORES  # heads per core

# Filler matmul counts bridging the head-0 fill phase (tuned on traces).
NF0 = 80  # pure warmup until head-0 q-lo lands (~13.9us: 6us SWDGE latency)
NF1 = 22  # between the Q-lo and K-lo transposes
NF2 = 38  # after the Q-hi transposes, while DVE ropes k-lo/q-hi


def rope_tables():
    """cosT/sinT in transposed layout [pair i, token t], bf16."""
    dim_half = DIM // 2
    inv = 1.0 / (10000.0 ** (np.arange(0, dim_half, 2).astype(np.float32) / dim_half))
    fx = np.outer(np.arange(GRID, dtype=np.float32), inv)  # (32, 64) by x
    fy = np.outer(np.arange(GRID, dtype=np.float32), inv)  # (32, 64) by y
    # token t = y*32 + x ; ang[t, i<64] = fx[x, i]; ang[t, i>=64] = fy[y, i-64]
    fx_grid = np.broadcast_to(fx[None, :, :], (GRID, GRID, fx.shape[1]))
    fy_grid = np.broadcast_to(fy[:, None, :], (GRID, GRID, fy.shape[1]))
    ang = np.concatenate([fx_grid, fy_grid], axis=-1).reshape(N, dim_half)
    cosT = np.ascontiguousarray(np.cos(ang).T).astype(ml_dtypes.bfloat16)
    sinT = np.ascontiguousarray(np.sin(ang).T).astype(ml_dtypes.bfloat16)
    return cosT, sinT


def build(n_heads=HPC):
    """Build the Bass program for one core processing n_heads heads."""
    import concourse.mybir as mybir
    import concourse.tile as tile
    from concourse import bacc
    from concourse.masks import make_identity

    bf16 = mybir.dt.bfloat16
    f32 = mybir.dt.float32
    Exp = mybir.ActivationFunctionType.Exp

    nc = bacc.Bacc(None, target_bir_lowering=False)
    names = {}

    with tile.TileContext(nc) as tc:
        with tc.tile_pool(name="dram", bufs=1, space="DRAM") as dram:
            Qd = dram.tile([n_heads, N, DIM], f32, kind="ExternalInput", name="Q")
            Kd = dram.tile([n_heads, N, DIM], f32, kind="ExternalInput", name="K")
            Vd = dram.tile([n_heads, N, DIM], f32, kind="ExternalInput", name="V")
            Cd = dram.tile([PAIRS, N], bf16, kind="ExternalInput", name="COS")
            Sd = dram.tile([PAIRS, N], bf16, kind="ExternalInput", name="SIN")
            Od = dram.tile([n_heads, N, DIM], f32, kind="ExternalOutput", name="OUT")
        names = {k: v.name for k, v in
                 dict(Q=Qd, K=Kd, V=Vd, COS=Cd, SIN=Sd, OUT=Od).items()}

        with (
            tc.tile_pool(name="const", bufs=1) as constp,
            tc.tile_pool(name="nat", bufs=12) as natp,
            tc.tile_pool(name="rt", bufs=6) as rtp,
            tc.tile_pool(name="tmp", bufs=4) as tmpp,
            tc.tile_pool(name="pt", bufs=3) as ptp,
            tc.tile_pool(name="va", bufs=3) as vap,
            tc.tile_pool(name="osb", bufs=3) as osbp,
            tc.tile_pool(name="rcp", bufs=8) as rcpp,
            tc.tile_pool(name="ptr", bufs=1, space="PSUM") as trp,
            tc.tile_pool(name="pst", bufs=3, space="PSUM") as stp,
            tc.tile_pool(name="pov", bufs=3, space="PSUM") as povp,
        ):
            # identity for PE transposes: first thing on gpsimd so the
            # head-0 cast-DMA issues right behind it
            ident = constp.tile([128, 128], bf16, name="ident")
            make_identity(nc, ident)

            NBH = NB // 2  # token blocks per load chunk

            def load_chunk(src, h, c):
                """Issue one half-tensor cast-DMA; returns pair-component
                view [p, nb, i, two]."""
                view = src[h].rearrange("(c nb p) d -> p c nb d", p=128, c=2)
                t = natp.tile([128, NBH, DIM], bf16, name="nh", tag="nat")
                nc.gpsimd.dma_start(t, view[:, c])
                return t

            def load_v(h):
                va = vap.tile([128, NB, DIM + 1], bf16, name="va", tag="va")
                nc.gpsimd.dma_start(
                    va[:, :, 0:DIM], Vd[h].rearrange("(mb p) d -> p mb d", p=128)
                )
                nc.vector.memset(va[:, :, DIM : DIM + 1], 1.0)
                return va

            # head-0 Q/K chunk loads go out first in consumption order
            # (q-lo, k-lo, q-hi, k-hi); every gpsimd cast-DMA completes
            # ~6us after issue (fixed SWDGE latency), contention or not.
            # cos/sin ride the otherwise-idle scalar HWDGE queue -- on the
            # sync queue behind other traffic they arrive ~16us and gate
            # the entire rope chain.
            q_chunks = [None, None]
            k_chunks = [None, None]
            q_chunks[0] = load_chunk(Qd, 0, 0)
            k_chunks[0] = load_chunk(Kd, 0, 0)
            q_chunks[1] = load_chunk(Qd, 0, 1)
            k_chunks[1] = load_chunk(Kd, 0, 1)
            va0 = load_v(0)

            cosT = constp.tile([128, N], bf16, name="cosT")
            sinT = constp.tile([128, N], bf16, name="sinT")
            nc.scalar.dma_start(cosT, Cd[:])
            nc.scalar.dma_start(sinT, Sd[:])

            # filler data for HAM-warming matmuls (DVE memset, t~0).
            # wupsum shares the st rotation: all fillers finish before the
            # first score matmul needs that bank (WAR dep, by construction).
            wudata = constp.tile([128, 128], bf16, name="wudata")
            nc.vector.memset(wudata, 0.5)
            wupsum = stp.tile([128, 512], f32, name="wupsum", tag="st")

            def filler(n, lhs=None, tag=""):
                """n dense 128-free matmuls on the PE. lhs (SBUF bf16
                [128, >=128]) pins placement: the block can't start before
                lhs's producer (DMA) lands, so emission order holds."""
                src = lhs if lhs is not None else wudata
                for i in range(n):
                    nc.tensor.matmul(
                        wupsum[:, 0:128],
                        lhsT=src[:, 0:128] if lhs is not None else wudata,
                        rhs=wudata,
                        start=(i == 0),
                        stop=(i == n - 1),
                    )

            def pair_view(t):
                return t.rearrange("p nb (i two) -> p two nb i", two=2)

            def transpose_chunk(dei, tr_pair, col_base):
                """Transpose one chunk's 4 token blocks, both halves, into
                trA/trB columns [col_base : col_base+512]."""
                trA, trB = tr_pair
                for half in (0, 1):
                    tr = trA if half == 0 else trB
                    for nb in range(NBH):
                        cb = col_base + nb * 128
                        nc.tensor.transpose(
                            tr[:, cb : cb + 128],
                            dei[:, half, nb],
                            ident,
                        )

            def rope_slice(rt, tr_pair, src_sl, tok_sl):
                """Apply rotary: rt[:, {0,1}, tok_sl] from trA/trB[:, src_sl]
                (tr columns may hold a different token window than tok_sl)."""
                trA, trB = tr_pair
                W = tok_sl.stop - tok_sl.start
                t1 = tmpp.tile([128, W], bf16, name="t1", tag="tmp")
                t2 = tmpp.tile([128, W], bf16, name="t2", tag="tmp")
                t3 = tmpp.tile([128, W], bf16, name="t3", tag="tmp")
                t4 = tmpp.tile([128, W], bf16, name="t4", tag="tmp")
                nc.vector.tensor_mul(t1, trA[:, src_sl], cosT[:, tok_sl])
                nc.vector.tensor_mul(t2, trB[:, src_sl], sinT[:, tok_sl])
                nc.vector.tensor_sub(rt[:, 0, tok_sl], t1, t2)
                nc.vector.tensor_mul(t3, trA[:, src_sl], sinT[:, tok_sl])
                nc.vector.tensor_mul(t4, trB[:, src_sl], cosT[:, tok_sl])
                nc.vector.tensor_add(rt[:, 1, tok_sl], t3, t4)

            def rope_tensor(dei_chunks, out_name):
                """Steady-state: transpose both chunks then rope full width."""
                rt = rtp.tile([128, 2, N], bf16, name=out_name, tag="rt")
                trA = trp.tile([128, N], bf16, name="trA", tag="trA")
                trB = trp.tile([128, N], bf16, name="trB", tag="trB")
                for c in range(2):
                    transpose_chunk(pair_view(dei_chunks[c]), (trA, trB), c * 512)
                rope_slice(rt, (trA, trB), slice(0, N), slice(0, N))
                return rt

            def scores_quarter(st_tiles, ptile, qrt, krt, nch, mbs):
                """Score matmuls + exp for mb in mbs, one n-chunk."""
                for mb in mbs:
                    st = st_tiles[(nch, mb)] = stp.tile(
                        [128, 512], f32, name="st", tag="st"
                    )
                    for dt_ in (0, 1):
                        nc.tensor.matmul(
                            st,
                            lhsT=krt[:, dt_, mb * 128 : (mb + 1) * 128],
                            rhs=qrt[:, dt_, nch * 512 : (nch + 1) * 512],
                            start=(dt_ == 0),
                            stop=(dt_ == 1),
                        )
                    nc.scalar.activation(
                        ptile[nch][:, mb],
                        st,
                        Exp,
                        scale=1.0 / 16.0,
                    )

            def pv_phase(h, ptiles, va, last):
                osb = osbp.tile([128, NB, DIM], f32, name="osb", tag="osb")
                od_view = Od[h].rearrange("(nb p) d -> p nb d", p=128)
                for nch in range(2):
                    ptile = ptiles[nch]
                    for nb4 in range(4):
                        po = povp.tile([128, DIM + 1], f32, name="po", tag="po")
                        for mb in range(NB):
                            nc.tensor.matmul(
                                po,
                                lhsT=ptile[:, mb, nb4 * 128 : (nb4 + 1) * 128],
                                rhs=va[:, mb],
                                start=(mb == 0),
                                stop=(mb == NB - 1),
                            )
                        r = rcpp.tile([128, 1], f32, name="r", tag="r")
                        nc.vector.reciprocal(r, po[:, DIM : DIM + 1])
                        gnb = nch * 4 + nb4
                        nc.vector.tensor_scalar_mul(osb[:, gnb], po[:, 0:DIM], r)
                        # finer stores on the last head shrink the tail drain
                        if last and gnb >= 6:
                            nc.sync.dma_start(
                                od_view[:, gnb : gnb + 1],
                                osb[:, gnb : gnb + 1],
                            )
                    if not last:
                        # store per chunk so the final transfer is half-sized
                        nc.sync.dma_start(
                            od_view[:, nch * 4 : (nch + 1) * 4],
                            osb[:, nch * 4 : (nch + 1) * 4],
                        )
                    elif nch == 0:
                        nc.sync.dma_start(od_view[:, 0:4], osb[:, 0:4])
                    else:
                        nc.sync.dma_start(od_view[:, 4:6], osb[:, 4:6])

            def attention(h, qrt, krt, va, split0=False):
                """Scores+exp then PV.  split0 emits head-0's scores in
                availability order (rope order: q-lo, k-lo, k-hi, q-hi)."""
                ptiles = [
                    ptp.tile([128, NB, 512], bf16, name="ptile", tag="pt")
                    for _ in range(2)
                ]
                st_tiles = {}
                if split0:
                    # availability order under rope q-lo, k-lo, q-hi, k-hi
                    scores_quarter(st_tiles, ptiles, qrt, krt, 0, range(0, 4))
                    scores_quarter(st_tiles, ptiles, qrt, krt, 1, range(0, 4))
                    scores_quarter(st_tiles, ptiles, qrt, krt, 0, range(4, 8))
                    scores_quarter(st_tiles, ptiles, qrt, krt, 1, range(4, 8))
                else:
                    scores_quarter(st_tiles, ptiles, qrt, krt, 0, range(8))
                    scores_quarter(st_tiles, ptiles, qrt, krt, 1, range(8))
                pv_phase(h, ptiles, va, last=(h == n_heads - 1))

            # ---- head 0: hand-scheduled fill ----
            # trA/trB columns are shared: [0:512] holds the current Q token
            # chunk, [512:1024] the current K token chunk.  Chunk 1's
            # transposes WAR-wait on chunk 0's rope reads, which is exactly
            # the availability order anyway.
            qrt0 = rtp.tile([128, 2, N], bf16, name="qrt", tag="rt")
            krt0 = rtp.tile([128, 2, N], bf16, name="krt", tag="rt")
            tr0 = (
                trp.tile([128, N], bf16, name="trA", tag="trA"),
                trp.tile([128, N], bf16, name="trB", tag="trB"),
            )
            # DVE rope order: q-lo, k-lo, q-hi, k-hi; hi-chunk transposes
            # WAR-wait on the lo rope reads of the shared tr columns, so
            # each rope_slice must be emitted before the transpose that
            # recycles its source columns.
            filler(NF0)
            transpose_chunk(pair_view(q_chunks[0]), tr0, 0)
            filler(NF1, lhs=q_chunks[0][:, 0, 0:128])
            transpose_chunk(pair_view(k_chunks[0]), tr0, 512)
            rope_slice(qrt0, tr0, slice(0, 512), slice(0, 512))
            rope_slice(krt0, tr0, slice(512, 1024), slice(0, 512))
            transpose_chunk(pair_view(q_chunks[1]), tr0, 0)
            filler(NF2, lhs=q_chunks[1][:, 0, 0:128])
            rope_slice(qrt0, tr0, slice(0, 512), slice(512, 1024))
            transpose_chunk(pair_view(k_chunks[1]), tr0, 512)
            rope_slice(krt0, tr0, slice(512, 1024), slice(512, 1024))

            pending = (0, qrt0, krt0, va0, True)

            # ---- heads 1..n-1: steady-state pipeline ----
            for h in range(1, n_heads + 1):
                if h < n_heads:
                    qc = [load_chunk(Qd, h, c) for c in range(2)]
                    kc = [load_chunk(Kd, h, c) for c in range(2)]
                    va = load_v(h)
                    qrt = rope_tensor(qc, "qrt")
                    krt = rope_tensor(kc, "krt")
                    cur = (h, qrt, krt, va, False)
                else:
                    cur = None
                if pending is not None:
                    attention(*pending[:4], split0=pending[4])
                pending = cur
            if pending is not None:
                attention(*pending[:4], split0=pending[4])

    nc.compile()
    return nc, names


_CACHE = {}


def _get_nc(n_heads=HPC):
    if n_heads not in _CACHE:
        _CACHE[n_heads] = build(n_heads)
    return _CACHE[n_heads]


def _run(Q, K, V, **spmd_kwargs):
    from concourse.bass_utils import run_bass_kernel_spmd

    nc, names = _get_nc(HPC)
    cosT, sinT = rope_tables()
    Qr = np.ascontiguousarray(Q.reshape(B * H, N, DIM), dtype=np.float32)
    Kr = np.ascontiguousarray(K.reshape(B * H, N, DIM), dtype=np.float32)
    Vr = np.ascontiguousarray(V.reshape(B * H, N, DIM), dtype=np.float32)
    in_maps = []
    for c in range(NCORES):
        sl = slice(c * HPC, (c + 1) * HPC)
        in_maps.append(
            {
                names["Q"]: np.ascontiguousarray(Qr[sl]),
                names["K"]: np.ascontiguousarray(Kr[sl]),
                names["V"]: np.ascontiguousarray(Vr[sl]),
                names["COS"]: cosT,
                names["SIN"]: sinT,
            }
        )
    res = run_bass_kernel_spmd(nc, in_maps, core_ids=list(range(NCORES)), **spmd_kwargs)
    out = np.concatenate([r[names["OUT"]] for r in res.results], axis=0)
    return np.ascontiguousarray(out.reshape(B, H, N, DIM), dtype=np.float32), res


def kernel(Q, K, V):
    return _run(Q, K, V)[0]


if __name__ == "__main__":
    rng = np.random.default_rng(0)
    Q = rng.standard_normal((B, H, N, DIM), dtype=np.float32)
    K = rng.standard_normal((B, H, N, DIM), dtype=np.float32)
    V = rng.standard_normal((B, H, N, DIM), dtype=np.float32)
    out = kernel(Q, K, V)
    print("out", out.shape, out.dtype, float(np.abs(out).mean()))
